# revision 1
# baseline (speedup 1.0000x reference)
"""EdgeUpdate (gnn_message_passing) Trainium2 Bass kernel — 8 NeuronCores.

Contract: kernel(**inputs) takes the FULL inputs of reference.setup_inputs()
and returns the FULL [32768, 64] float32 output.

Strategy (edge-parallel SPMD):
- Edges are sharded 8 ways (4096 edges/core); node features and all
  FFN/LN parameters are replicated. One NEFF runs on cores 0-7 with
  per-core input bindings. No cross-core communication.

Per-core kernel (tiles of 128 edges, natural layout: edges on partitions):
- src/dst node rows gathered from res_features by indirect DMA.
- FFN1 computed transposed (rhT = relu(W1.T @ efT + b1)) so the bias is
  per-partition; efT is host-pretransposed and DMA'd directly.
- FFN2 (the [128]x[128,5120] per-edge weight generation) runs in
  float32r (full PE rate at N>=256, ~1.6e-4 rel err) into PSUM chunks,
  with W2 host-permuted h-major (col h*80+d).
- The per-edge tensor-product contraction tp[h] = sum_d feats[d]*w[d,h]
  is one fused pass over each PSUM chunk using a custom DVE op
  (MUL_SCAN: running prefix sum of Src0*Src1, chained across chunks via
  a per-partition init scalar); tp is recovered from the prefix samples
  at page boundaries by a strided copy + difference.
- b2 enters via a small feats @ b2.reshape(80,64) matmul (featsT from a
  PE transpose); residual + LayerNorm close the tile.
"""

from contextlib import ExitStack

import numpy as np

import concourse.bass as bass
import concourse.dve_ops as dve_ops
import concourse.tile as tile
from concourse import bacc, mybir
from concourse.bass_utils import run_bass_kernel_spmd
from concourse.dve_spec import AluOp, C0, Spec, Src0, Src1, lower, scan
from concourse.dve_uop import DveOpSpec
from concourse.masks import make_identity

F32 = mybir.dt.float32
F32R = mybir.dt.float32r
N_CORES = 8
E_TOTAL = 32768
E_CORE = E_TOTAL // N_CORES
N_NODES = 16384
RES_DIM = 56
H = 64
W_IN = 80
W_NUMEL = 5120
EPS = 1e-5
CHUNK_PAGES = 16  # h-pages (of 80 values) per PSUM chunk
PRELOAD = 2


def _register_mul_scan():
    name = "MUL_SCAN_ANT"
    for op in dve_ops.OPS:
        if op.name == name:
            return op
    spec = Spec(
        body=scan(AluOp.ADD, Src0 * Src1, init=C0),
        reference=lambda in0, in1, s0, *a: (
            np.asarray(s0, np.float32)
            + np.cumsum(
                (in0.reshape(in0.shape[0], -1)
                 * in1.reshape(in1.shape[0], -1)).astype(np.float32),
                axis=-1)).astype(np.float32),
    )
    opcode = dve_ops._CUSTOM_DVE_ROW_BASE + len(dve_ops.OPS)
    shas = {}
    for ver in ("v3", "v4"):
        shas[ver] = DveOpSpec(name=name, opcode=opcode,
                              uops=lower(spec, ver=ver), rd1_en=True).sha(ver)
    op = dve_ops.DveOp(name, spec, subdim=False, uops_sha=shas)
    dve_ops.OPS.append(op)
    dve_ops.CUSTOM_DVE_SPECS[name] = spec
    dve_ops._SUB_OPCODE_FOR_NAME[name] = opcode
    return op


MUL_SCAN = _register_mul_scan()


def _build_kernel():
    n_tiles = E_CORE // 128
    chunks = []
    h = 0
    while h < H:
        n = min(CHUNK_PAGES, H - h)
        chunks.append((h, n))
        h += n

    nc = bacc.Bacc("TRN2", target_bir_lowering=False, debug=False,
                   enable_asserts=False, num_devices=N_CORES)

    efsi_ap = nc.dram_tensor("efsi", [E_CORE, H + 6], F32,
                             kind="ExternalInput").ap()
    eft_ap = nc.dram_tensor("eft", [H, E_CORE], F32, kind="ExternalInput").ap()
    res_ap = nc.dram_tensor("res", [N_NODES, RES_DIM], F32,
                            kind="ExternalInput").ap()
    w1_ap = nc.dram_tensor("w1", [H, 128], F32, kind="ExternalInput").ap()
    b1_ap = nc.dram_tensor("b1", [128, 1], F32, kind="ExternalInput").ap()
    w2_ap = nc.dram_tensor("w2", [128, W_NUMEL], F32R, kind="ExternalInput").ap()
    b2r_ap = nc.dram_tensor("b2r", [W_IN, H], F32, kind="ExternalInput").ap()
    gb_ap = nc.dram_tensor("gb", [2, H], F32, kind="ExternalInput").ap()
    out_ap = nc.dram_tensor("out", [E_CORE, H], F32, kind="ExternalOutput").ap()

    with tile.TileContext(nc) as tc, ExitStack() as ctx:
        singles = ctx.enter_context(tc.tile_pool(name="singles", bufs=1))
        loads = ctx.enter_context(tc.tile_pool(name="loads", bufs=5))
        mids = ctx.enter_context(tc.tile_pool(name="mids", bufs=3))
        scanp = ctx.enter_context(tc.tile_pool(name="scanp", bufs=2))
        outs = ctx.enter_context(tc.tile_pool(name="outs", bufs=3))
        ps_small = ctx.enter_context(
            tc.tile_pool(name="ps_small", bufs=1, space="PSUM"))
        ps_rh = ctx.enter_context(
            tc.tile_pool(name="ps_rh", bufs=1, space="PSUM"))
        ps_w = ctx.enter_context(tc.tile_pool(name="ps_w", bufs=2, space="PSUM"))

        # --- resident constants ---
        w1sb = singles.tile([H, 128], F32)
        nc.sync.dma_start(w1sb[:], w1_ap[:])
        b1sb = singles.tile([128, 1], F32)
        nc.sync.dma_start(b1sb[:], b1_ap[:])
        b2rsb = singles.tile([W_IN, H], F32)
        nc.sync.dma_start(b2rsb[:], b2r_ap[:])
        gammab = singles.tile([128, H], F32)
        nc.sync.dma_start(gammab[:], bass.AP(
            tensor=gb_ap.tensor, offset=gb_ap.offset,
            ap=[[0, 128]] + gb_ap[0:1, :].ap[1:]))
        betab = singles.tile([128, H], F32)
        nc.sync.dma_start(betab[:], bass.AP(
            tensor=gb_ap.tensor, offset=gb_ap.offset + H,
            ap=[[0, 128]] + gb_ap[1:2, :].ap[1:]))
        epsb = singles.tile([128, 1], F32)
        nc.vector.memset(epsb[:], EPS)
        ident = singles.tile([128, 128], F32)
        make_identity(nc, ident[:])

        w2blocks = []
        nblk = 4
        blkw = W_NUMEL // nblk

        def load_w2(blocks):
            for bi in blocks:
                w2b = singles.tile([128, blkw], F32R, tag=f"w2b{bi}")
                nc.scalar.dma_start(w2b[:], w2_ap[:, bi * blkw:(bi + 1) * blkw])
                w2blocks.append(w2b)

        def w2_slice(c0, c1):
            bi = c0 // blkw
            assert c1 <= (bi + 1) * blkw
            return w2blocks[bi][:, c0 - bi * blkw:c1 - bi * blkw]

        def emit_loads(it):
            rows = slice(it * 128, (it + 1) * 128)
            efsi = loads.tile([128, H + 6], F32, tag="efsi")
            nc.sync.dma_start(efsi[:], efsi_ap[rows, :])
            ef = efsi[:, 0:H]
            sh = efsi[:, H:H + 4]
            idx = efsi[:, H + 4:H + 6].bitcast(mybir.dt.int32)
            src = loads.tile([128, RES_DIM], F32, tag="gath")
            dst = loads.tile([128, RES_DIM], F32, tag="gath")
            nc.gpsimd.indirect_dma_start(
                out=src[:], out_offset=None, in_=res_ap[:],
                in_offset=bass.IndirectOffsetOnAxis(ap=idx[:, 0:1], axis=0))
            nc.gpsimd.indirect_dma_start(
                out=dst[:], out_offset=None, in_=res_ap[:],
                in_offset=bass.IndirectOffsetOnAxis(ap=idx[:, 1:2], axis=0))
            efT = mids.tile([H, 128], F32, tag="efT")
            nc.sync.dma_start(efT[:], eft_ap[:, rows])
            return ef, sh, idx, src, dst, efT

        load_w2([0])
        pre = {it: emit_loads(it) for it in range(min(PRELOAD, n_tiles))}
        load_w2([1, 2, 3])

        for it in range(n_tiles):
            rows = slice(it * 128, (it + 1) * 128)
            ef, sh, idx, src, dst, efT = (
                pre[it] if it in pre else emit_loads(it))

            # --- FFN1, transposed out: rhT = relu(W1.T @ efT + b1) ---
            rhT_ps = ps_rh.tile([128, 128], F32, tag="psr")
            nc.tensor.matmul(rhT_ps[:], lhsT=w1sb[:], rhs=efT[:],
                             start=True, stop=True)
            rhT = mids.tile([128, 128], F32R)
            nc.scalar.activation(rhT[:], rhT_ps[:],
                                 mybir.ActivationFunctionType.Relu,
                                 bias=b1sb[:], scale=1.0)

            # --- feats [128, 80] ---
            feats = mids.tile([128, W_IN], F32)
            nc.scalar.activation(feats[:, 0:32], src[:, 0:32],
                                 mybir.ActivationFunctionType.Copy,
                                 scale=sh[:, 0:1])
            nc.scalar.activation(feats[:, 32:64], dst[:, 0:32],
                                 mybir.ActivationFunctionType.Copy,
                                 scale=sh[:, 0:1])
            vtmp = mids.tile([128, 8], F32)
            for half, g in ((src, slice(64, 72)), (dst, slice(72, 80))):
                vecs = half[:, 32:RES_DIM].rearrange("p (m c) -> p m c", c=3)
                nc.gpsimd.tensor_scalar_mul(feats[:, g], in0=vecs[:, :, 0],
                                            scalar1=sh[:, 1:2])
                for comp in (1, 2):
                    nc.gpsimd.tensor_scalar_mul(
                        vtmp[:], in0=vecs[:, :, comp],
                        scalar1=sh[:, 1 + comp:2 + comp])
                    nc.gpsimd.tensor_tensor(out=feats[:, g], in0=feats[:, g],
                                            in1=vtmp[:],
                                            op=mybir.AluOpType.add)

            # --- b2 seed: feats @ b2r via PE transpose of feats ---
            featsT_ps = ps_small.tile([W_IN, 128], F32, tag="ps")
            nc.tensor.transpose(featsT_ps[:], feats[:], ident[:])
            featsT = mids.tile([W_IN, 128], F32)
            nc.scalar.copy(featsT[:], featsT_ps[:])
            seed_ps = ps_small.tile([128, H], F32, tag="ps")
            nc.tensor.matmul(seed_ps[:], lhsT=featsT[:], rhs=b2rsb[:],
                             start=True, stop=True)
            seedc = mids.tile([128, H], F32)
            nc.scalar.copy(seedc[:], seed_ps[:])
            x_pre = outs.tile([128, H], F32)
            nc.gpsimd.tensor_tensor(out=x_pre[:], in0=ef[:], in1=seedc[:],
                                    op=mybir.AluOpType.add)

            # --- FFN2 (f32r) + fused scan contraction ---
            junk = scanp.tile([128, W_NUMEL], F32)
            for ci, (h0, npages) in enumerate(chunks):
                width = npages * W_IN
                w_ps = ps_w.tile([128, width], F32, tag="psw")
                col = 0
                while col < width:
                    n = min(512, width - col)
                    nc.tensor.matmul(
                        w_ps[:, col:col + n], lhsT=rhT[:],
                        rhs=w2_slice(h0 * W_IN + col, h0 * W_IN + col + n),
                        start=True, stop=True)
                    col += n
                init = 0.0 if ci == 0 else junk[:, h0 * W_IN - 1:h0 * W_IN]
                feats_b = bass.AP(tensor=feats[:].tensor,
                                  offset=feats[:].offset,
                                  ap=[feats[:].ap[0], [0, npages], [1, W_IN]])
                nc.vector._custom_dve(
                    MUL_SCAN,
                    out=junk[:, h0 * W_IN:(h0 + npages) * W_IN].rearrange(
                        "p (s n) -> p s n", n=W_IN),
                    in0=w_ps[:].rearrange("p (s n) -> p s n", n=W_IN),
                    in1=feats_b, s0=init)

            # tp[h] = S[h] - S[h-1] where S[h] = prefix at end of page h
            S = outs.tile([128, H], F32)
            nc.vector.tensor_copy(
                S[:], junk[:].rearrange("p (s n) -> p s n", n=W_IN)[:, :, 79])
            x = outs.tile([128, H], F32)
            nc.gpsimd.tensor_tensor(out=x[:, 0:1], in0=x_pre[:, 0:1],
                                    in1=S[:, 0:1], op=mybir.AluOpType.add)
            d1 = outs.tile([128, H - 1], F32)
            nc.vector.tensor_tensor(out=d1[:], in0=S[:, 1:], in1=S[:, :H - 1],
                                    op=mybir.AluOpType.subtract)
            nc.gpsimd.tensor_tensor(out=x[:, 1:], in0=x_pre[:, 1:],
                                    in1=d1[:], op=mybir.AluOpType.add)

            # --- LayerNorm ---
            stats = outs.tile([128, 6], F32)
            nc.vector.bn_stats(out=stats[:], in_=x[:])
            mv = outs.tile([128, 2], F32)
            nc.vector.bn_aggr(out=mv[:], in_=stats[:])
            std = outs.tile([128, 1], F32)
            nc.scalar.activation(std[:], mv[:, 1:2],
                                 mybir.ActivationFunctionType.Sqrt,
                                 bias=epsb[:], scale=1.0)
            rstd = outs.tile([128, 1], F32)
            nc.vector.reciprocal(rstd[:], std[:])
            y = outs.tile([128, H], F32)
            nc.gpsimd.tensor_scalar(out=y[:], in0=x[:], scalar1=mv[:, 0:1],
                                    scalar2=rstd[:],
                                    op0=mybir.AluOpType.subtract,
                                    op1=mybir.AluOpType.mult)
            nc.gpsimd.tensor_tensor(out=y[:], in0=y[:], in1=gammab[:],
                                    op=mybir.AluOpType.mult)
            nc.gpsimd.tensor_tensor(out=y[:], in0=y[:], in1=betab[:],
                                    op=mybir.AluOpType.add)
            nc.sync.dma_start(out_ap[rows, :], y[:])

    nc.compile()
    return nc


_NC_CACHE = None


def _get_nc():
    global _NC_CACHE
    if _NC_CACHE is None:
        _NC_CACHE = _build_kernel()
    return _NC_CACHE


def _host_prep(inputs):
    ef = np.asarray(inputs["edge_features"], np.float32)
    sh = np.asarray(inputs["edge_sh"], np.float32).copy()
    sh[:, 1:4] /= np.float32(np.sqrt(3.0))
    idx = np.asarray(inputs["edge_index"])
    res = np.ascontiguousarray(np.asarray(inputs["res_features"], np.float32))
    w1 = np.ascontiguousarray(np.asarray(inputs["W1"], np.float32))
    b1 = np.ascontiguousarray(
        np.asarray(inputs["b1"], np.float32).reshape(128, 1))
    scale = np.float32(1.0 / np.sqrt(80.0))
    w2 = np.asarray(inputs["W2"], np.float32) * scale
    # h-major permutation: col h*80+d = w2[:, d*64+h]
    w2 = np.ascontiguousarray(
        w2.reshape(128, W_IN, H).transpose(0, 2, 1).reshape(128, W_NUMEL))
    b2r = np.ascontiguousarray(
        (np.asarray(inputs["b2"], np.float32) * scale).reshape(W_IN, H))
    gb = np.ascontiguousarray(np.stack([
        np.asarray(inputs["gamma"], np.float32),
        np.asarray(inputs["beta"], np.float32)]))
    # idx cols: 0 = src (edge_index[1]), 1 = dst (edge_index[0]), as int32
    idx2 = np.stack([idx[1], idx[0]], axis=1).astype(np.int32)

    in_maps = []
    for c in range(N_CORES):
        rows = slice(c * E_CORE, (c + 1) * E_CORE)
        efsi = np.concatenate(
            [ef[rows], sh[rows], idx2[rows].view(np.float32)], axis=1)
        in_maps.append(dict(
            efsi=np.ascontiguousarray(efsi),
            eft=np.ascontiguousarray(ef[rows].T),
            res=res, w1=w1, b1=b1, w2=w2, b2r=b2r, gb=gb,
        ))
    return in_maps


def kernel(**inputs) -> np.ndarray:
    assert inputs["edge_features"].shape == (E_TOTAL, H)
    nc = _get_nc()
    in_maps = _host_prep(inputs)
    res = run_bass_kernel_spmd(nc, in_maps, core_ids=list(range(N_CORES)))
    return np.concatenate([r["out"] for r in res.results], axis=0)


# revision 2
# speedup vs baseline: 1.0027x; 1.0027x over previous
"""EdgeUpdate (gnn_message_passing) Trainium2 Bass kernel — 8 NeuronCores.

Contract: kernel(**inputs) takes the FULL inputs of reference.setup_inputs()
and returns the FULL [32768, 64] float32 output.

Strategy (edge-parallel SPMD):
- Edges are sharded 8 ways (4096 edges/core); node features and all
  FFN/LN parameters are replicated. One NEFF runs on cores 0-7 with
  per-core input bindings. No cross-core communication.

Per-core kernel (tiles of 128 edges, natural layout: edges on partitions):
- src/dst node rows gathered from res_features by indirect DMA (indices
  packed int32 into the per-tile input block, bitcast on chip).
- FFN1 computed transposed (rhT = relu(W1.T @ efT + b1)) so the bias is
  per-partition; efT is host-pretransposed and DMA'd directly.
- FFN2 (the [128]x[128,5120] per-edge weight generation) runs in
  float32r (full PE rate at N>=256, ~1.6e-4 rel err) into 16-h-page
  PSUM chunks, with W2 host-permuted h-major (col h*80+d).
- The per-edge tensor-product contraction tp[h] = sum_d feats[d]*w[d,h]
  is one fused DVE pass per PSUM chunk using a custom op (MUL_SCAN:
  running prefix sum of Src0*Src1, chained across chunks via a
  per-partition init scalar). The output access pattern has a stride-0
  innermost dim, so the 80 per-page prefix writes collapse onto one
  address and the chunk directly deposits the page-end samples S[h];
  tp falls out as first differences of S.
- b2 enters via a small feats @ b2.reshape(80,64) matmul (featsT from a
  PE transpose); residual + LayerNorm close the tile. LN-tail emission
  is software-pipelined one tile behind the scans to keep the DVE FIFO
  free of cross-engine waits.
"""

from contextlib import ExitStack

import numpy as np

import concourse.bass as bass
import concourse.dve_ops as dve_ops
import concourse.tile as tile
from concourse import bacc, mybir
from concourse.bass_utils import run_bass_kernel_spmd
from concourse.dve_spec import AluOp, C0, Spec, Src0, Src1, lower, scan
from concourse.dve_uop import DveOpSpec
from concourse.masks import make_identity

F32 = mybir.dt.float32
F32R = mybir.dt.float32r
N_CORES = 8
E_TOTAL = 32768
E_CORE = E_TOTAL // N_CORES
N_NODES = 16384
RES_DIM = 56
H = 64
W_IN = 80
W_NUMEL = 5120
EPS = 1e-5
CHUNK_PAGES = 16  # h-pages (of 80 values) per PSUM chunk
PRELOAD = 2


def _register_mul_scan():
    name = "MUL_SCAN_ANT"
    for op in dve_ops.OPS:
        if op.name == name:
            return op
    spec = Spec(
        body=scan(AluOp.ADD, Src0 * Src1, init=C0),
        reference=lambda in0, in1, s0, *a: (
            np.asarray(s0, np.float32)
            + np.cumsum(
                (in0.reshape(in0.shape[0], -1)
                 * in1.reshape(in1.shape[0], -1)).astype(np.float32),
                axis=-1)).astype(np.float32),
    )
    opcode = dve_ops._CUSTOM_DVE_ROW_BASE + len(dve_ops.OPS)
    shas = {}
    for ver in ("v3", "v4"):
        shas[ver] = DveOpSpec(name=name, opcode=opcode,
                              uops=lower(spec, ver=ver), rd1_en=True).sha(ver)
    op = dve_ops.DveOp(name, spec, subdim=False, uops_sha=shas)
    dve_ops.OPS.append(op)
    dve_ops.CUSTOM_DVE_SPECS[name] = spec
    dve_ops._SUB_OPCODE_FOR_NAME[name] = opcode
    return op


MUL_SCAN = _register_mul_scan()


def _build_kernel():
    n_tiles = E_CORE // 128
    chunks = []
    h = 0
    while h < H:
        n = min(CHUNK_PAGES, H - h)
        chunks.append((h, n))
        h += n

    nc = bacc.Bacc("TRN2", target_bir_lowering=False, debug=False,
                   enable_asserts=False, num_devices=N_CORES)

    efsi_ap = nc.dram_tensor("efsi", [E_CORE, H + 6], F32,
                             kind="ExternalInput").ap()
    eft_ap = nc.dram_tensor("eft", [H, E_CORE], F32, kind="ExternalInput").ap()
    res_ap = nc.dram_tensor("res", [N_NODES, RES_DIM], F32,
                            kind="ExternalInput").ap()
    w1_ap = nc.dram_tensor("w1", [H, 128], F32, kind="ExternalInput").ap()
    b1_ap = nc.dram_tensor("b1", [128, 1], F32, kind="ExternalInput").ap()
    w2_ap = nc.dram_tensor("w2", [128, W_NUMEL], F32R, kind="ExternalInput").ap()
    b2r_ap = nc.dram_tensor("b2r", [W_IN, H], F32, kind="ExternalInput").ap()
    gb_ap = nc.dram_tensor("gb", [2, H], F32, kind="ExternalInput").ap()
    out_ap = nc.dram_tensor("out", [E_CORE, H], F32, kind="ExternalOutput").ap()

    with tile.TileContext(nc) as tc, ExitStack() as ctx:
        singles = ctx.enter_context(tc.tile_pool(name="singles", bufs=1))
        loads = ctx.enter_context(tc.tile_pool(name="loads", bufs=5))
        mids = ctx.enter_context(tc.tile_pool(name="mids", bufs=3))
        outs = ctx.enter_context(tc.tile_pool(name="outs", bufs=3))
        ps_small = ctx.enter_context(
            tc.tile_pool(name="ps_small", bufs=1, space="PSUM"))
        ps_rh = ctx.enter_context(
            tc.tile_pool(name="ps_rh", bufs=1, space="PSUM"))
        ps_w = ctx.enter_context(tc.tile_pool(name="ps_w", bufs=2, space="PSUM"))

        # --- resident constants ---
        w1sb = singles.tile([H, 128], F32)
        nc.sync.dma_start(w1sb[:], w1_ap[:])
        b1sb = singles.tile([128, 1], F32)
        nc.sync.dma_start(b1sb[:], b1_ap[:])
        b2rsb = singles.tile([W_IN, H], F32)
        nc.sync.dma_start(b2rsb[:], b2r_ap[:])
        gammab = singles.tile([128, H], F32)
        nc.sync.dma_start(gammab[:], bass.AP(
            tensor=gb_ap.tensor, offset=gb_ap.offset,
            ap=[[0, 128]] + gb_ap[0:1, :].ap[1:]))
        betab = singles.tile([128, H], F32)
        nc.sync.dma_start(betab[:], bass.AP(
            tensor=gb_ap.tensor, offset=gb_ap.offset + H,
            ap=[[0, 128]] + gb_ap[1:2, :].ap[1:]))
        epsb = singles.tile([128, 1], F32)
        nc.vector.memset(epsb[:], EPS)
        ident = singles.tile([128, 128], F32)
        make_identity(nc, ident[:])

        w2blocks = []
        nblk = 4
        blkw = W_NUMEL // nblk

        def load_w2(blocks):
            for bi in blocks:
                w2b = singles.tile([128, blkw], F32R, tag=f"w2b{bi}")
                nc.scalar.dma_start(w2b[:], w2_ap[:, bi * blkw:(bi + 1) * blkw])
                w2blocks.append(w2b)

        def w2_slice(c0, c1):
            bi = c0 // blkw
            assert c1 <= (bi + 1) * blkw
            return w2blocks[bi][:, c0 - bi * blkw:c1 - bi * blkw]

        def emit_loads(it):
            rows = slice(it * 128, (it + 1) * 128)
            efsi = loads.tile([128, H + 6], F32, tag="efsi")
            nc.sync.dma_start(efsi[:], efsi_ap[rows, :])
            ef = efsi[:, 0:H]
            sh = efsi[:, H:H + 4]
            idx = efsi[:, H + 4:H + 6].bitcast(mybir.dt.int32)
            src = loads.tile([128, RES_DIM], F32, tag="gath")
            dst = loads.tile([128, RES_DIM], F32, tag="gath")
            nc.gpsimd.indirect_dma_start(
                out=src[:], out_offset=None, in_=res_ap[:],
                in_offset=bass.IndirectOffsetOnAxis(ap=idx[:, 0:1], axis=0))
            nc.gpsimd.indirect_dma_start(
                out=dst[:], out_offset=None, in_=res_ap[:],
                in_offset=bass.IndirectOffsetOnAxis(ap=idx[:, 1:2], axis=0))
            efT = mids.tile([H, 128], F32, tag="efT")
            nc.sync.dma_start(efT[:], eft_ap[:, rows])
            return ef, sh, idx, src, dst, efT

        def emit_tail(rows, S, x_pre):
            x = outs.tile([128, H], F32, tag="x")
            nc.vector.tensor_tensor(out=x[:, 0:1], in0=x_pre[:, 0:1],
                                    in1=S[:, 0:1], op=mybir.AluOpType.add)
            d1 = outs.tile([128, H - 1], F32, tag="d1")
            nc.vector.tensor_tensor(out=d1[:], in0=S[:, 1:], in1=S[:, :H - 1],
                                    op=mybir.AluOpType.subtract)
            nc.vector.tensor_tensor(out=x[:, 1:], in0=x_pre[:, 1:],
                                    in1=d1[:], op=mybir.AluOpType.add)
            stats = outs.tile([128, 6], F32, tag="stats")
            nc.vector.bn_stats(out=stats[:], in_=x[:])
            mv = outs.tile([128, 2], F32, tag="mv")
            nc.vector.bn_aggr(out=mv[:], in_=stats[:])
            std = outs.tile([128, 1], F32, tag="std")
            nc.scalar.activation(std[:], mv[:, 1:2],
                                 mybir.ActivationFunctionType.Sqrt,
                                 bias=epsb[:], scale=1.0)
            rstd = outs.tile([128, 1], F32, tag="rstd")
            nc.vector.reciprocal(rstd[:], std[:])
            y = outs.tile([128, H], F32, tag="y")
            nc.gpsimd.tensor_scalar(out=y[:], in0=x[:], scalar1=mv[:, 0:1],
                                    scalar2=rstd[:],
                                    op0=mybir.AluOpType.subtract,
                                    op1=mybir.AluOpType.mult)
            nc.gpsimd.tensor_tensor(out=y[:], in0=y[:], in1=gammab[:],
                                    op=mybir.AluOpType.mult)
            nc.gpsimd.tensor_tensor(out=y[:], in0=y[:], in1=betab[:],
                                    op=mybir.AluOpType.add)
            nc.sync.dma_start(out_ap[rows, :], y[:])

        load_w2([0])
        pre = {it: emit_loads(it) for it in range(min(PRELOAD, n_tiles))}
        load_w2([1, 2, 3])
        pending = None

        for it in range(n_tiles):
            rows = slice(it * 128, (it + 1) * 128)
            ef, sh, idx, src, dst, efT = (
                pre[it] if it in pre else emit_loads(it))

            # --- FFN1, transposed out: rhT = relu(W1.T @ efT + b1) ---
            rhT_ps = ps_rh.tile([128, 128], F32, tag="psr")
            nc.tensor.matmul(rhT_ps[:], lhsT=w1sb[:], rhs=efT[:],
                             start=True, stop=True)
            rhT = mids.tile([128, 128], F32R)
            nc.scalar.activation(rhT[:], rhT_ps[:],
                                 mybir.ActivationFunctionType.Relu,
                                 bias=b1sb[:], scale=1.0)

            # --- feats [128, 80] ---
            feats = mids.tile([128, W_IN], F32)
            nc.scalar.activation(feats[:, 0:32], src[:, 0:32],
                                 mybir.ActivationFunctionType.Copy,
                                 scale=sh[:, 0:1])
            nc.scalar.activation(feats[:, 32:64], dst[:, 0:32],
                                 mybir.ActivationFunctionType.Copy,
                                 scale=sh[:, 0:1])
            vtmp = mids.tile([128, 8], F32)
            for half, g in ((src, slice(64, 72)), (dst, slice(72, 80))):
                vecs = half[:, 32:RES_DIM].rearrange("p (m c) -> p m c", c=3)
                nc.gpsimd.tensor_scalar_mul(feats[:, g], in0=vecs[:, :, 0],
                                            scalar1=sh[:, 1:2])
                for comp in (1, 2):
                    nc.gpsimd.tensor_scalar_mul(
                        vtmp[:], in0=vecs[:, :, comp],
                        scalar1=sh[:, 1 + comp:2 + comp])
                    nc.gpsimd.tensor_tensor(out=feats[:, g], in0=feats[:, g],
                                            in1=vtmp[:],
                                            op=mybir.AluOpType.add)

            # --- b2 seed: feats @ b2r via PE transpose of feats ---
            featsT_ps = ps_small.tile([W_IN, 128], F32, tag="ps")
            nc.tensor.transpose(featsT_ps[:], feats[:], ident[:])
            featsT = mids.tile([W_IN, 128], F32)
            nc.scalar.copy(featsT[:], featsT_ps[:])
            seed_ps = ps_small.tile([128, H], F32, tag="ps")
            nc.tensor.matmul(seed_ps[:], lhsT=featsT[:], rhs=b2rsb[:],
                             start=True, stop=True)
            seedc = mids.tile([128, H], F32)
            nc.scalar.copy(seedc[:], seed_ps[:])
            x_pre = outs.tile([128, H], F32, tag="x_pre")
            nc.gpsimd.tensor_tensor(out=x_pre[:], in0=ef[:], in1=seedc[:],
                                    op=mybir.AluOpType.add)

            # --- FFN2 (f32r) + fused scan contraction into S ---
            S = outs.tile([128, H], F32, tag="S")
            for ci, (h0, npages) in enumerate(chunks):
                width = npages * W_IN
                w_ps = ps_w.tile([128, width], F32, tag="psw")
                col = 0
                while col < width:
                    n = min(512, width - col)
                    nc.tensor.matmul(
                        w_ps[:, col:col + n], lhsT=rhT[:],
                        rhs=w2_slice(h0 * W_IN + col, h0 * W_IN + col + n),
                        start=True, stop=True)
                    col += n
                init = 0.0 if ci == 0 else S[:, h0 - 1:h0]
                feats_b = bass.AP(tensor=feats[:].tensor,
                                  offset=feats[:].offset,
                                  ap=[feats[:].ap[0], [0, npages], [1, W_IN]])
                s_out = bass.AP(tensor=S[:].tensor, offset=S[:].offset + h0,
                                ap=[S[:].ap[0], [1, npages], [0, W_IN]])
                nc.vector._custom_dve(
                    MUL_SCAN, out=s_out,
                    in0=w_ps[:].rearrange("p (s n) -> p s n", n=W_IN),
                    in1=feats_b, s0=init)

            if pending is not None:
                emit_tail(*pending)
            pending = (rows, S, x_pre)

        if pending is not None:
            emit_tail(*pending)

    nc.compile()
    return nc


_NC_CACHE = None


def _get_nc():
    global _NC_CACHE
    if _NC_CACHE is None:
        _NC_CACHE = _build_kernel()
    return _NC_CACHE


def _host_prep(inputs):
    ef = np.asarray(inputs["edge_features"], np.float32)
    sh = np.asarray(inputs["edge_sh"], np.float32).copy()
    sh[:, 1:4] /= np.float32(np.sqrt(3.0))
    idx = np.asarray(inputs["edge_index"])
    res = np.ascontiguousarray(np.asarray(inputs["res_features"], np.float32))
    w1 = np.ascontiguousarray(np.asarray(inputs["W1"], np.float32))
    b1 = np.ascontiguousarray(
        np.asarray(inputs["b1"], np.float32).reshape(128, 1))
    scale = np.float32(1.0 / np.sqrt(80.0))
    w2 = np.asarray(inputs["W2"], np.float32) * scale
    # h-major permutation: col h*80+d = w2[:, d*64+h]
    w2 = np.ascontiguousarray(
        w2.reshape(128, W_IN, H).transpose(0, 2, 1).reshape(128, W_NUMEL))
    b2r = np.ascontiguousarray(
        (np.asarray(inputs["b2"], np.float32) * scale).reshape(W_IN, H))
    gb = np.ascontiguousarray(np.stack([
        np.asarray(inputs["gamma"], np.float32),
        np.asarray(inputs["beta"], np.float32)]))
    # idx cols: 0 = src (edge_index[1]), 1 = dst (edge_index[0]), as int32
    idx2 = np.stack([idx[1], idx[0]], axis=1).astype(np.int32)

    in_maps = []
    for c in range(N_CORES):
        rows = slice(c * E_CORE, (c + 1) * E_CORE)
        efsi = np.concatenate(
            [ef[rows], sh[rows], idx2[rows].view(np.float32)], axis=1)
        in_maps.append(dict(
            efsi=np.ascontiguousarray(efsi),
            eft=np.ascontiguousarray(ef[rows].T),
            res=res, w1=w1, b1=b1, w2=w2, b2r=b2r, gb=gb,
        ))
    return in_maps


def kernel(**inputs) -> np.ndarray:
    assert inputs["edge_features"].shape == (E_TOTAL, H)
    nc = _get_nc()
    in_maps = _host_prep(inputs)
    res = run_bass_kernel_spmd(nc, in_maps, core_ids=list(range(N_CORES)))
    return np.concatenate([r["out"] for r in res.results], axis=0)


# revision 3
# speedup vs baseline: 1.0205x; 1.0177x over previous
"""EdgeUpdate (gnn_message_passing) Trainium2 Bass kernel — 8 NeuronCores.

Contract: kernel(**inputs) takes the FULL inputs of reference.setup_inputs()
and returns the FULL [32768, 64] float32 output.

Strategy (edge-parallel SPMD):
- Edges are sharded 8 ways (4096 edges/core); node features and all
  FFN/LN parameters are replicated. One NEFF runs on cores 0-7 with
  per-core input bindings. No cross-core communication.

Per-core kernel (tiles of 128 edges, natural layout: edges on partitions):
- src/dst node rows gathered from res_features by indirect DMA (indices
  packed int32 into the per-tile input block, bitcast on chip).
- FFN1 computed transposed (rhT = relu(W1.T @ efT + b1)) so the bias is
  per-partition; efT is host-pretransposed and DMA'd directly.
- FFN2 (the [128]x[128,5120] per-edge weight generation) runs in
  float32r (full PE rate at N>=256, ~1.6e-4 rel err) into 16-h-page
  PSUM chunks, with W2 host-permuted h-major (col h*80+d).
- The per-edge tensor-product contraction tp[h] = sum_d feats[d]*w[d,h]
  is one fused DVE pass per PSUM chunk using a custom op (MUL_SCAN:
  running prefix sum of Src0*Src1, chained across chunks via a
  per-partition init scalar). The output access pattern has a stride-0
  innermost dim, so the 80 per-page prefix writes collapse onto one
  address and the chunk directly deposits the page-end samples S[h];
  tp falls out as first differences of S.
- b2 enters via a small feats @ b2.reshape(80,64) matmul (featsT from a
  PE transpose); residual + LayerNorm close the tile. LN-tail emission
  is software-pipelined one tile behind the scans to keep the DVE FIFO
  free of cross-engine waits.
"""

from contextlib import ExitStack

import numpy as np

import concourse.bass as bass
import concourse.dve_ops as dve_ops
import concourse.tile as tile
from concourse import bacc, mybir
from concourse.bass_utils import run_bass_kernel_spmd
from concourse.dve_spec import AluOp, C0, Spec, Src0, Src1, lower, scan
from concourse.dve_uop import DveOpSpec
from concourse.masks import make_identity

F32 = mybir.dt.float32
F32R = mybir.dt.float32r
N_CORES = 8
E_TOTAL = 32768
E_CORE = E_TOTAL // N_CORES
N_NODES = 16384
RES_DIM = 56
H = 64
W_IN = 80
W_NUMEL = 5120
EPS = 1e-5
CHUNK_PAGES = 16  # h-pages (of 80 values) per PSUM chunk
PRELOAD = 1


def _register_mul_scan():
    name = "MUL_SCAN_ANT"
    for op in dve_ops.OPS:
        if op.name == name:
            return op
    spec = Spec(
        body=scan(AluOp.ADD, Src0 * Src1, init=C0),
        reference=lambda in0, in1, s0, *a: (
            np.asarray(s0, np.float32)
            + np.cumsum(
                (in0.reshape(in0.shape[0], -1)
                 * in1.reshape(in1.shape[0], -1)).astype(np.float32),
                axis=-1)).astype(np.float32),
    )
    opcode = dve_ops._CUSTOM_DVE_ROW_BASE + len(dve_ops.OPS)
    shas = {}
    for ver in ("v3", "v4"):
        shas[ver] = DveOpSpec(name=name, opcode=opcode,
                              uops=lower(spec, ver=ver), rd1_en=True).sha(ver)
    op = dve_ops.DveOp(name, spec, subdim=False, uops_sha=shas)
    dve_ops.OPS.append(op)
    dve_ops.CUSTOM_DVE_SPECS[name] = spec
    dve_ops._SUB_OPCODE_FOR_NAME[name] = opcode
    return op


MUL_SCAN = _register_mul_scan()


def _build_kernel():
    n_tiles = E_CORE // 128
    chunks = []
    h = 0
    while h < H:
        n = min(CHUNK_PAGES, H - h)
        chunks.append((h, n))
        h += n

    nc = bacc.Bacc("TRN2", target_bir_lowering=False, debug=False,
                   enable_asserts=False, num_devices=N_CORES)

    efsi_ap = nc.dram_tensor("efsi", [E_CORE, H + 6], F32,
                             kind="ExternalInput").ap()
    eft_ap = nc.dram_tensor("eft", [H, E_CORE], F32, kind="ExternalInput").ap()
    res_ap = nc.dram_tensor("res", [N_NODES, RES_DIM], F32,
                            kind="ExternalInput").ap()
    w1_ap = nc.dram_tensor("w1", [H, 128], F32, kind="ExternalInput").ap()
    b1_ap = nc.dram_tensor("b1", [128, 1], F32, kind="ExternalInput").ap()
    w2_ap = nc.dram_tensor("w2", [128, W_NUMEL], F32R, kind="ExternalInput").ap()
    b2r_ap = nc.dram_tensor("b2r", [W_IN, H], F32, kind="ExternalInput").ap()
    gb_ap = nc.dram_tensor("gb", [2, H], F32, kind="ExternalInput").ap()
    out_ap = nc.dram_tensor("out", [E_CORE, H], F32, kind="ExternalOutput").ap()

    with tile.TileContext(nc) as tc, ExitStack() as ctx:
        singles = ctx.enter_context(tc.tile_pool(name="singles", bufs=1))
        loads = ctx.enter_context(tc.tile_pool(name="loads", bufs=3))
        mids = ctx.enter_context(tc.tile_pool(name="mids", bufs=3))
        outs = ctx.enter_context(tc.tile_pool(name="outs", bufs=3))
        ps_small = ctx.enter_context(
            tc.tile_pool(name="ps_small", bufs=1, space="PSUM"))
        ps_rh = ctx.enter_context(
            tc.tile_pool(name="ps_rh", bufs=1, space="PSUM"))
        ps_w = ctx.enter_context(tc.tile_pool(name="ps_w", bufs=2, space="PSUM"))

        # --- resident constants ---
        w1sb = singles.tile([H, 128], F32)
        nc.sync.dma_start(w1sb[:], w1_ap[:])
        b1sb = singles.tile([128, 1], F32)
        nc.sync.dma_start(b1sb[:], b1_ap[:])
        b2rsb = singles.tile([W_IN, H], F32)
        nc.sync.dma_start(b2rsb[:], b2r_ap[:])
        gammab = singles.tile([128, H], F32)
        nc.sync.dma_start(gammab[:], bass.AP(
            tensor=gb_ap.tensor, offset=gb_ap.offset,
            ap=[[0, 128]] + gb_ap[0:1, :].ap[1:]))
        betab = singles.tile([128, H], F32)
        nc.sync.dma_start(betab[:], bass.AP(
            tensor=gb_ap.tensor, offset=gb_ap.offset + H,
            ap=[[0, 128]] + gb_ap[1:2, :].ap[1:]))
        epsb = singles.tile([128, 1], F32)
        nc.vector.memset(epsb[:], EPS)
        ident = singles.tile([128, 128], F32)
        make_identity(nc, ident[:])

        w2blocks = []
        nblk = 4
        blkw = W_NUMEL // nblk

        def load_w2(blocks):
            for bi in blocks:
                w2b = singles.tile([128, blkw], F32R, tag=f"w2b{bi}")
                nc.scalar.dma_start(w2b[:], w2_ap[:, bi * blkw:(bi + 1) * blkw])
                w2blocks.append(w2b)

        def w2_slice(c0, c1):
            bi = c0 // blkw
            assert c1 <= (bi + 1) * blkw
            return w2blocks[bi][:, c0 - bi * blkw:c1 - bi * blkw]

        def emit_loads(it, eng=None):
            eng = eng or nc.sync
            rows = slice(it * 128, (it + 1) * 128)
            efsi = loads.tile([128, H + 6], F32, tag="efsi")
            eng.dma_start(efsi[:], efsi_ap[rows, :])
            ef = efsi[:, 0:H]
            sh = efsi[:, H:H + 4]
            idx = efsi[:, H + 4:H + 6].bitcast(mybir.dt.int32)
            src = loads.tile([128, RES_DIM], F32, tag="gath")
            dst = loads.tile([128, RES_DIM], F32, tag="gath")
            nc.gpsimd.indirect_dma_start(
                out=src[:], out_offset=None, in_=res_ap[:],
                in_offset=bass.IndirectOffsetOnAxis(ap=idx[:, 0:1], axis=0))
            nc.gpsimd.indirect_dma_start(
                out=dst[:], out_offset=None, in_=res_ap[:],
                in_offset=bass.IndirectOffsetOnAxis(ap=idx[:, 1:2], axis=0))
            efT = mids.tile([H, 128], F32, tag="efT")
            eng.dma_start(efT[:], eft_ap[:, rows])
            return ef, sh, idx, src, dst, efT

        def emit_tail(rows, S, x_pre):
            x = outs.tile([128, H], F32, tag="x")
            nc.vector.tensor_tensor(out=x[:], in0=x_pre[:], in1=S[:],
                                    op=mybir.AluOpType.add)
            nc.vector.tensor_tensor(out=x[:, 1:], in0=x[:, 1:],
                                    in1=S[:, :H - 1],
                                    op=mybir.AluOpType.subtract)
            stats = outs.tile([128, 6], F32, tag="stats")
            nc.vector.bn_stats(out=stats[:], in_=x[:])
            mv = outs.tile([128, 2], F32, tag="mv")
            nc.vector.bn_aggr(out=mv[:], in_=stats[:])
            std = outs.tile([128, 1], F32, tag="std")
            nc.scalar.activation(std[:], mv[:, 1:2],
                                 mybir.ActivationFunctionType.Sqrt,
                                 bias=epsb[:], scale=1.0)
            rstd = outs.tile([128, 1], F32, tag="rstd")
            nc.vector.reciprocal(rstd[:], std[:])
            y = outs.tile([128, H], F32, tag="y")
            nc.gpsimd.tensor_scalar(out=y[:], in0=x[:], scalar1=mv[:, 0:1],
                                    scalar2=rstd[:],
                                    op0=mybir.AluOpType.subtract,
                                    op1=mybir.AluOpType.mult)
            nc.gpsimd.tensor_tensor(out=y[:], in0=y[:], in1=gammab[:],
                                    op=mybir.AluOpType.mult)
            nc.gpsimd.tensor_tensor(out=y[:], in0=y[:], in1=betab[:],
                                    op=mybir.AluOpType.add)
            nc.sync.dma_start(out_ap[rows, :], y[:])

        load_w2([0])
        # preload the first tile's inputs via the Pool SWDGE path with top
        # priority so they don't queue behind the W2 spray on the HW queues
        with tc.high_priority():
            pre = {it: emit_loads(it, nc.gpsimd)
                   for it in range(min(PRELOAD, n_tiles))}
        load_w2([1, 2, 3])
        pending = None

        for it in range(n_tiles):
            rows = slice(it * 128, (it + 1) * 128)
            ef, sh, idx, src, dst, efT = (
                pre[it] if it in pre else emit_loads(it))

            # --- FFN1, transposed out: rhT = relu(W1.T @ efT + b1) ---
            rhT_ps = ps_rh.tile([128, 128], F32, tag="psr")
            nc.tensor.matmul(rhT_ps[:], lhsT=w1sb[:], rhs=efT[:],
                             start=True, stop=True)
            rhT = mids.tile([128, 128], F32R)
            nc.scalar.activation(rhT[:], rhT_ps[:],
                                 mybir.ActivationFunctionType.Relu,
                                 bias=b1sb[:], scale=1.0)

            # --- feats [128, 80] ---
            feats = mids.tile([128, W_IN], F32)
            nc.scalar.activation(feats[:, 0:32], src[:, 0:32],
                                 mybir.ActivationFunctionType.Copy,
                                 scale=sh[:, 0:1])
            nc.scalar.activation(feats[:, 32:64], dst[:, 0:32],
                                 mybir.ActivationFunctionType.Copy,
                                 scale=sh[:, 0:1])
            vtmp = mids.tile([128, 8], F32)
            for half, g in ((src, slice(64, 72)), (dst, slice(72, 80))):
                vecs = half[:, 32:RES_DIM].rearrange("p (m c) -> p m c", c=3)
                nc.gpsimd.tensor_scalar_mul(feats[:, g], in0=vecs[:, :, 0],
                                            scalar1=sh[:, 1:2])
                for comp in (1, 2):
                    nc.gpsimd.tensor_scalar_mul(
                        vtmp[:], in0=vecs[:, :, comp],
                        scalar1=sh[:, 1 + comp:2 + comp])
                    nc.gpsimd.tensor_tensor(out=feats[:, g], in0=feats[:, g],
                                            in1=vtmp[:],
                                            op=mybir.AluOpType.add)

            # --- b2 seed: feats @ b2r via PE transpose of feats ---
            featsT_ps = ps_small.tile([W_IN, 128], F32, tag="ps")
            nc.tensor.transpose(featsT_ps[:], feats[:], ident[:])
            featsT = mids.tile([W_IN, 128], F32)
            nc.scalar.copy(featsT[:], featsT_ps[:])
            seed_ps = ps_small.tile([128, H], F32, tag="ps")
            nc.tensor.matmul(seed_ps[:], lhsT=featsT[:], rhs=b2rsb[:],
                             start=True, stop=True)
            seedc = mids.tile([128, H], F32)
            nc.scalar.copy(seedc[:], seed_ps[:])
            x_pre = outs.tile([128, H], F32, tag="x_pre")
            nc.gpsimd.tensor_tensor(out=x_pre[:], in0=ef[:], in1=seedc[:],
                                    op=mybir.AluOpType.add)

            # --- FFN2 (f32r) + fused scan contraction into S ---
            S = outs.tile([128, H], F32, tag="S")
            for ci, (h0, npages) in enumerate(chunks):
                width = npages * W_IN
                w_ps = ps_w.tile([128, width], F32, tag="psw")
                col = 0
                while col < width:
                    n = min(512, width - col)
                    nc.tensor.matmul(
                        w_ps[:, col:col + n], lhsT=rhT[:],
                        rhs=w2_slice(h0 * W_IN + col, h0 * W_IN + col + n),
                        start=True, stop=True)
                    col += n
                init = 0.0 if ci == 0 else S[:, h0 - 1:h0]
                feats_b = bass.AP(tensor=feats[:].tensor,
                                  offset=feats[:].offset,
                                  ap=[feats[:].ap[0], [0, npages], [1, W_IN]])
                s_out = bass.AP(tensor=S[:].tensor, offset=S[:].offset + h0,
                                ap=[S[:].ap[0], [1, npages], [0, W_IN]])
                nc.vector._custom_dve(
                    MUL_SCAN, out=s_out,
                    in0=w_ps[:].rearrange("p (s n) -> p s n", n=W_IN),
                    in1=feats_b, s0=init)

            if pending is not None:
                emit_tail(*pending)
            pending = (rows, S, x_pre)

        if pending is not None:
            emit_tail(*pending)

    nc.compile()
    return nc


_NC_CACHE = None


def _get_nc():
    global _NC_CACHE
    if _NC_CACHE is None:
        _NC_CACHE = _build_kernel()
    return _NC_CACHE


def _host_prep(inputs):
    ef = np.asarray(inputs["edge_features"], np.float32)
    sh = np.asarray(inputs["edge_sh"], np.float32).copy()
    sh[:, 1:4] /= np.float32(np.sqrt(3.0))
    idx = np.asarray(inputs["edge_index"])
    res = np.ascontiguousarray(np.asarray(inputs["res_features"], np.float32))
    w1 = np.ascontiguousarray(np.asarray(inputs["W1"], np.float32))
    b1 = np.ascontiguousarray(
        np.asarray(inputs["b1"], np.float32).reshape(128, 1))
    scale = np.float32(1.0 / np.sqrt(80.0))
    w2 = np.asarray(inputs["W2"], np.float32) * scale
    # h-major permutation: col h*80+d = w2[:, d*64+h]
    w2 = np.ascontiguousarray(
        w2.reshape(128, W_IN, H).transpose(0, 2, 1).reshape(128, W_NUMEL))
    b2r = np.ascontiguousarray(
        (np.asarray(inputs["b2"], np.float32) * scale).reshape(W_IN, H))
    gb = np.ascontiguousarray(np.stack([
        np.asarray(inputs["gamma"], np.float32),
        np.asarray(inputs["beta"], np.float32)]))
    # idx cols: 0 = src (edge_index[1]), 1 = dst (edge_index[0]), as int32
    idx2 = np.stack([idx[1], idx[0]], axis=1).astype(np.int32)

    in_maps = []
    for c in range(N_CORES):
        rows = slice(c * E_CORE, (c + 1) * E_CORE)
        efsi = np.concatenate(
            [ef[rows], sh[rows], idx2[rows].view(np.float32)], axis=1)
        in_maps.append(dict(
            efsi=np.ascontiguousarray(efsi),
            eft=np.ascontiguousarray(ef[rows].T),
            res=res, w1=w1, b1=b1, w2=w2, b2r=b2r, gb=gb,
        ))
    return in_maps


def kernel(**inputs) -> np.ndarray:
    assert inputs["edge_features"].shape == (E_TOTAL, H)
    nc = _get_nc()
    in_maps = _host_prep(inputs)
    res = run_bass_kernel_spmd(nc, in_maps, core_ids=list(range(N_CORES)))
    return np.concatenate([r["out"] for r in res.results], axis=0)


# revision 4
# speedup vs baseline: 1.0337x; 1.0129x over previous
"""EdgeUpdate (gnn_message_passing) Trainium2 Bass kernel — 8 NeuronCores.

Contract: kernel(**inputs) takes the FULL inputs of reference.setup_inputs()
and returns the FULL [32768, 64] float32 output.

Strategy (edge-parallel SPMD):
- Edges are sharded 8 ways (4096 edges/core); node features and all
  FFN/LN parameters are replicated. One NEFF runs on cores 0-7 with
  per-core input bindings. No cross-core communication.

Per-core kernel (tiles of 128 edges, natural layout: edges on partitions):
- src/dst node rows gathered from res_features by indirect DMA (indices
  packed int32 into the per-tile input block, bitcast on chip).
- FFN1 computed transposed (rhT = relu(W1.T @ efT + b1)) so the bias is
  per-partition; efT is host-pretransposed and DMA'd directly.
- FFN2 (the [128]x[128,5120] per-edge weight generation) runs in
  float32r (full PE rate at N>=256, ~1.6e-4 rel err) into 16-h-page
  PSUM chunks, with W2 host-permuted h-major (col h*80+d).
- The per-edge tensor-product contraction tp[h] = sum_d feats[d]*w[d,h]
  is one fused DVE pass per PSUM chunk using a custom op (MUL_SCAN:
  running prefix sum of Src0*Src1, chained across chunks via a
  per-partition init scalar). The output access pattern has a stride-0
  innermost dim, so the 80 per-page prefix writes collapse onto one
  address and the chunk directly deposits the page-end samples S[h];
  tp falls out as first differences of S.
- b2 enters via a small feats @ b2.reshape(80,64) matmul (featsT from a
  PE transpose); residual + LayerNorm close the tile. LN-tail emission
  is software-pipelined one tile behind the scans to keep the DVE FIFO
  free of cross-engine waits.
"""

from contextlib import ExitStack

import numpy as np

import concourse.bass as bass
import concourse.dve_ops as dve_ops
import concourse.tile as tile
from concourse import bacc, mybir
from concourse.bass_utils import run_bass_kernel_spmd
from concourse.dve_spec import AluOp, C0, Spec, Src0, Src1, lower, scan
from concourse.dve_uop import DveOpSpec
from concourse.masks import make_identity

F32 = mybir.dt.float32
F32R = mybir.dt.float32r
N_CORES = 8
E_TOTAL = 32768
E_CORE = E_TOTAL // N_CORES
N_NODES = 16384
RES_DIM = 56
H = 64
W_IN = 80
W_NUMEL = 5120
EPS = 1e-5
CHUNK_PAGES = 16  # h-pages (of 80 values) per PSUM chunk
PRELOAD = 1


def _register_mul_scan():
    name = "MUL_SCAN_ANT"
    for op in dve_ops.OPS:
        if op.name == name:
            return op
    spec = Spec(
        body=scan(AluOp.ADD, Src0 * Src1, init=C0),
        reference=lambda in0, in1, s0, *a: (
            np.asarray(s0, np.float32)
            + np.cumsum(
                (in0.reshape(in0.shape[0], -1)
                 * in1.reshape(in1.shape[0], -1)).astype(np.float32),
                axis=-1)).astype(np.float32),
    )
    opcode = dve_ops._CUSTOM_DVE_ROW_BASE + len(dve_ops.OPS)
    shas = {}
    for ver in ("v3", "v4"):
        shas[ver] = DveOpSpec(name=name, opcode=opcode,
                              uops=lower(spec, ver=ver), rd1_en=True).sha(ver)
    op = dve_ops.DveOp(name, spec, subdim=False, uops_sha=shas)
    dve_ops.OPS.append(op)
    dve_ops.CUSTOM_DVE_SPECS[name] = spec
    dve_ops._SUB_OPCODE_FOR_NAME[name] = opcode
    return op


MUL_SCAN = _register_mul_scan()


def _build_kernel():
    n_tiles = E_CORE // 128
    chunks = []
    h = 0
    while h < H:
        n = min(CHUNK_PAGES, H - h)
        chunks.append((h, n))
        h += n

    nc = bacc.Bacc("TRN2", target_bir_lowering=False, debug=False,
                   enable_asserts=False, num_devices=N_CORES)

    efsi_ap = nc.dram_tensor("efsi", [E_CORE, H + 6], F32,
                             kind="ExternalInput").ap()
    eft_ap = nc.dram_tensor("eft", [H, E_CORE], F32, kind="ExternalInput").ap()
    res_ap = nc.dram_tensor("res", [N_NODES, RES_DIM], F32,
                            kind="ExternalInput").ap()
    w1_ap = nc.dram_tensor("w1", [H, 128], F32, kind="ExternalInput").ap()
    b1_ap = nc.dram_tensor("b1", [128, 1], F32, kind="ExternalInput").ap()
    w2_ap = nc.dram_tensor("w2", [128, W_NUMEL], F32R, kind="ExternalInput").ap()
    b2r_ap = nc.dram_tensor("b2r", [W_IN, H], F32, kind="ExternalInput").ap()
    gb_ap = nc.dram_tensor("gb", [2, H], F32, kind="ExternalInput").ap()
    out_ap = nc.dram_tensor("out", [E_CORE, H], F32, kind="ExternalOutput").ap()

    with tile.TileContext(nc) as tc, ExitStack() as ctx:
        singles = ctx.enter_context(tc.tile_pool(name="singles", bufs=1))
        loads = ctx.enter_context(tc.tile_pool(name="loads", bufs=3))
        mids = ctx.enter_context(tc.tile_pool(name="mids", bufs=3))
        outs = ctx.enter_context(tc.tile_pool(name="outs", bufs=3))
        ps_small = ctx.enter_context(
            tc.tile_pool(name="ps_small", bufs=1, space="PSUM"))
        ps_rh = ctx.enter_context(
            tc.tile_pool(name="ps_rh", bufs=1, space="PSUM"))
        ps_w = ctx.enter_context(tc.tile_pool(name="ps_w", bufs=2, space="PSUM"))

        # --- resident constants ---
        w1sb = singles.tile([H, 128], F32)
        nc.sync.dma_start(w1sb[:], w1_ap[:])
        b1sb = singles.tile([128, 1], F32)
        nc.sync.dma_start(b1sb[:], b1_ap[:])
        b2rsb = singles.tile([W_IN, H], F32)
        nc.sync.dma_start(b2rsb[:], b2r_ap[:])
        gammab = singles.tile([128, H], F32)
        nc.sync.dma_start(gammab[:], bass.AP(
            tensor=gb_ap.tensor, offset=gb_ap.offset,
            ap=[[0, 128]] + gb_ap[0:1, :].ap[1:]))
        betab = singles.tile([128, H], F32)
        nc.sync.dma_start(betab[:], bass.AP(
            tensor=gb_ap.tensor, offset=gb_ap.offset + H,
            ap=[[0, 128]] + gb_ap[1:2, :].ap[1:]))
        # scheduling nudge: one extra sync-queue DMA here shifts the DMA
        # queue interleave so tile-0's critical transfers land ~3us earlier
        scratch = singles.tile([1, W_NUMEL], F32R)
        nc.sync.dma_start(scratch[:], w2_ap[0:1, :])
        epsb = singles.tile([128, 1], F32)
        nc.vector.memset(epsb[:], EPS)
        ident = singles.tile([128, 128], F32)
        make_identity(nc, ident[:])

        w2blocks = []
        nblk = 4
        blkw = W_NUMEL // nblk

        def load_w2(blocks):
            for bi in blocks:
                w2b = singles.tile([128, blkw], F32R, tag=f"w2b{bi}")
                nc.scalar.dma_start(w2b[:], w2_ap[:, bi * blkw:(bi + 1) * blkw])
                w2blocks.append(w2b)

        def w2_slice(c0, c1):
            bi = c0 // blkw
            assert c1 <= (bi + 1) * blkw
            return w2blocks[bi][:, c0 - bi * blkw:c1 - bi * blkw]

        def emit_loads(it, eng=None):
            eng = eng or nc.sync
            rows = slice(it * 128, (it + 1) * 128)
            efsi = loads.tile([128, H + 6], F32, tag="efsi")
            eng.dma_start(efsi[:], efsi_ap[rows, :])
            ef = efsi[:, 0:H]
            sh = efsi[:, H:H + 4]
            idx = efsi[:, H + 4:H + 6].bitcast(mybir.dt.int32)
            src = loads.tile([128, RES_DIM], F32, tag="gath")
            dst = loads.tile([128, RES_DIM], F32, tag="gath")
            nc.gpsimd.indirect_dma_start(
                out=src[:], out_offset=None, in_=res_ap[:],
                in_offset=bass.IndirectOffsetOnAxis(ap=idx[:, 0:1], axis=0))
            nc.gpsimd.indirect_dma_start(
                out=dst[:], out_offset=None, in_=res_ap[:],
                in_offset=bass.IndirectOffsetOnAxis(ap=idx[:, 1:2], axis=0))
            efT = mids.tile([H, 128], F32, tag="efT")
            eng.dma_start(efT[:], eft_ap[:, rows])
            return ef, sh, idx, src, dst, efT

        def emit_tail(rows, S, x_pre):
            x = outs.tile([128, H], F32, tag="x")
            nc.vector.tensor_tensor(out=x[:], in0=x_pre[:], in1=S[:],
                                    op=mybir.AluOpType.add)
            nc.vector.tensor_tensor(out=x[:, 1:], in0=x[:, 1:],
                                    in1=S[:, :H - 1],
                                    op=mybir.AluOpType.subtract)
            stats = outs.tile([128, 6], F32, tag="stats")
            nc.vector.bn_stats(out=stats[:], in_=x[:])
            mv = outs.tile([128, 2], F32, tag="mv")
            nc.vector.bn_aggr(out=mv[:], in_=stats[:])
            std = outs.tile([128, 1], F32, tag="std")
            nc.scalar.activation(std[:], mv[:, 1:2],
                                 mybir.ActivationFunctionType.Sqrt,
                                 bias=epsb[:], scale=1.0)
            rstd = outs.tile([128, 1], F32, tag="rstd")
            nc.vector.reciprocal(rstd[:], std[:])
            y = outs.tile([128, H], F32, tag="y")
            nc.gpsimd.tensor_scalar(out=y[:], in0=x[:], scalar1=mv[:, 0:1],
                                    scalar2=rstd[:],
                                    op0=mybir.AluOpType.subtract,
                                    op1=mybir.AluOpType.mult)
            nc.gpsimd.tensor_tensor(out=y[:], in0=y[:], in1=gammab[:],
                                    op=mybir.AluOpType.mult)
            nc.gpsimd.tensor_tensor(out=y[:], in0=y[:], in1=betab[:],
                                    op=mybir.AluOpType.add)
            nc.sync.dma_start(out_ap[rows, :], y[:])

        load_w2([0])
        # preload the first tile's inputs via the Pool SWDGE path with top
        # priority so they don't queue behind the W2 spray on the HW queues
        with tc.high_priority():
            pre = {it: emit_loads(it, nc.gpsimd)
                   for it in range(min(PRELOAD, n_tiles))}
        load_w2([1, 2, 3])
        pending = None

        for it in range(n_tiles):
            rows = slice(it * 128, (it + 1) * 128)
            ef, sh, idx, src, dst, efT = (
                pre[it] if it in pre else emit_loads(it))

            # --- FFN1, transposed out: rhT = relu(W1.T @ efT + b1) ---
            rhT_ps = ps_rh.tile([128, 128], F32, tag="psr")
            nc.tensor.matmul(rhT_ps[:], lhsT=w1sb[:], rhs=efT[:],
                             start=True, stop=True)
            rhT = mids.tile([128, 128], F32R)
            nc.scalar.activation(rhT[:], rhT_ps[:],
                                 mybir.ActivationFunctionType.Relu,
                                 bias=b1sb[:], scale=1.0)

            # --- feats [128, 80] ---
            feats = mids.tile([128, W_IN], F32)
            nc.scalar.activation(feats[:, 0:32], src[:, 0:32],
                                 mybir.ActivationFunctionType.Copy,
                                 scale=sh[:, 0:1])
            nc.scalar.activation(feats[:, 32:64], dst[:, 0:32],
                                 mybir.ActivationFunctionType.Copy,
                                 scale=sh[:, 0:1])
            vtmp = mids.tile([128, 8], F32)
            for half, g in ((src, slice(64, 72)), (dst, slice(72, 80))):
                vecs = half[:, 32:RES_DIM].rearrange("p (m c) -> p m c", c=3)
                nc.gpsimd.tensor_scalar_mul(feats[:, g], in0=vecs[:, :, 0],
                                            scalar1=sh[:, 1:2])
                for comp in (1, 2):
                    nc.gpsimd.tensor_scalar_mul(
                        vtmp[:], in0=vecs[:, :, comp],
                        scalar1=sh[:, 1 + comp:2 + comp])
                    nc.gpsimd.tensor_tensor(out=feats[:, g], in0=feats[:, g],
                                            in1=vtmp[:],
                                            op=mybir.AluOpType.add)

            # --- b2 seed: feats @ b2r via PE transpose of feats ---
            featsT_ps = ps_small.tile([W_IN, 128], F32, tag="ps")
            nc.tensor.transpose(featsT_ps[:], feats[:], ident[:])
            featsT = mids.tile([W_IN, 128], F32)
            nc.scalar.copy(featsT[:], featsT_ps[:])
            seed_ps = ps_small.tile([128, H], F32, tag="ps")
            nc.tensor.matmul(seed_ps[:], lhsT=featsT[:], rhs=b2rsb[:],
                             start=True, stop=True)
            seedc = mids.tile([128, H], F32)
            nc.scalar.copy(seedc[:], seed_ps[:])
            x_pre = outs.tile([128, H], F32, tag="x_pre")
            nc.gpsimd.tensor_tensor(out=x_pre[:], in0=ef[:], in1=seedc[:],
                                    op=mybir.AluOpType.add)

            # --- FFN2 (f32r) + fused scan contraction into S ---
            S = outs.tile([128, H], F32, tag="S")
            for ci, (h0, npages) in enumerate(chunks):
                width = npages * W_IN
                w_ps = ps_w.tile([128, width], F32, tag="psw")
                col = 0
                while col < width:
                    n = min(512, width - col)
                    nc.tensor.matmul(
                        w_ps[:, col:col + n], lhsT=rhT[:],
                        rhs=w2_slice(h0 * W_IN + col, h0 * W_IN + col + n),
                        start=True, stop=True)
                    col += n
                init = 0.0 if ci == 0 else S[:, h0 - 1:h0]
                feats_b = bass.AP(tensor=feats[:].tensor,
                                  offset=feats[:].offset,
                                  ap=[feats[:].ap[0], [0, npages], [1, W_IN]])
                s_out = bass.AP(tensor=S[:].tensor, offset=S[:].offset + h0,
                                ap=[S[:].ap[0], [1, npages], [0, W_IN]])
                nc.vector._custom_dve(
                    MUL_SCAN, out=s_out,
                    in0=w_ps[:].rearrange("p (s n) -> p s n", n=W_IN),
                    in1=feats_b, s0=init)

            if pending is not None:
                emit_tail(*pending)
            pending = (rows, S, x_pre)

        if pending is not None:
            emit_tail(*pending)

    nc.compile()
    return nc


_NC_CACHE = None


def _get_nc():
    global _NC_CACHE
    if _NC_CACHE is None:
        _NC_CACHE = _build_kernel()
    return _NC_CACHE


def _host_prep(inputs):
    ef = np.asarray(inputs["edge_features"], np.float32)
    sh = np.asarray(inputs["edge_sh"], np.float32).copy()
    sh[:, 1:4] /= np.float32(np.sqrt(3.0))
    idx = np.asarray(inputs["edge_index"])
    res = np.ascontiguousarray(np.asarray(inputs["res_features"], np.float32))
    w1 = np.ascontiguousarray(np.asarray(inputs["W1"], np.float32))
    b1 = np.ascontiguousarray(
        np.asarray(inputs["b1"], np.float32).reshape(128, 1))
    scale = np.float32(1.0 / np.sqrt(80.0))
    w2 = np.asarray(inputs["W2"], np.float32) * scale
    # h-major permutation: col h*80+d = w2[:, d*64+h]
    w2 = np.ascontiguousarray(
        w2.reshape(128, W_IN, H).transpose(0, 2, 1).reshape(128, W_NUMEL))
    b2r = np.ascontiguousarray(
        (np.asarray(inputs["b2"], np.float32) * scale).reshape(W_IN, H))
    gb = np.ascontiguousarray(np.stack([
        np.asarray(inputs["gamma"], np.float32),
        np.asarray(inputs["beta"], np.float32)]))
    # idx cols: 0 = src (edge_index[1]), 1 = dst (edge_index[0]), as int32
    idx2 = np.stack([idx[1], idx[0]], axis=1).astype(np.int32)

    in_maps = []
    for c in range(N_CORES):
        rows = slice(c * E_CORE, (c + 1) * E_CORE)
        efsi = np.concatenate(
            [ef[rows], sh[rows], idx2[rows].view(np.float32)], axis=1)
        in_maps.append(dict(
            efsi=np.ascontiguousarray(efsi),
            eft=np.ascontiguousarray(ef[rows].T),
            res=res, w1=w1, b1=b1, w2=w2, b2r=b2r, gb=gb,
        ))
    return in_maps


def kernel(**inputs) -> np.ndarray:
    assert inputs["edge_features"].shape == (E_TOTAL, H)
    nc = _get_nc()
    in_maps = _host_prep(inputs)
    res = run_bass_kernel_spmd(nc, in_maps, core_ids=list(range(N_CORES)))
    return np.concatenate([r["out"] for r in res.results], axis=0)


# revision 5
# speedup vs baseline: 1.0386x; 1.0047x over previous
"""EdgeUpdate (gnn_message_passing) Trainium2 Bass kernel — 8 NeuronCores.

Contract: kernel(**inputs) takes the FULL inputs of reference.setup_inputs()
and returns the FULL [32768, 64] float32 output.

Strategy (edge-parallel SPMD):
- Edges are sharded 8 ways (4096 edges/core); node features and all
  FFN/LN parameters are replicated. One NEFF runs on cores 0-7 with
  per-core input bindings. No cross-core communication.

Per-core kernel (tiles of 128 edges, natural layout: edges on partitions):
- src/dst node rows gathered from res_features by indirect DMA (indices
  packed int32 into the per-tile input block, bitcast on chip).
- FFN1 computed transposed (rhT = relu(W1.T @ efT + b1)) so the bias is
  per-partition; efT is host-pretransposed and DMA'd directly.
- FFN2 (the [128]x[128,5120] per-edge weight generation) runs in
  float32r (full PE rate at N>=256, ~1.6e-4 rel err) into 16-h-page
  PSUM chunks, with W2 host-permuted h-major (col h*80+d).
- The per-edge tensor-product contraction tp[h] = sum_d feats[d]*w[d,h]
  is one fused DVE pass per PSUM chunk using a custom op (MUL_SCAN:
  running prefix sum of Src0*Src1, chained across chunks via a
  per-partition init scalar). The output access pattern has a stride-0
  innermost dim, so the 80 per-page prefix writes collapse onto one
  address and the chunk directly deposits the page-end samples S[h];
  tp falls out as first differences of S.
- b2 enters via a small feats @ b2.reshape(80,64) matmul (featsT from a
  PE transpose); residual + LayerNorm close the tile. LN-tail emission
  is software-pipelined one tile behind the scans to keep the DVE FIFO
  free of cross-engine waits.
"""

from contextlib import ExitStack

import numpy as np

import concourse.bass as bass
import concourse.dve_ops as dve_ops
import concourse.tile as tile
from concourse import bacc, mybir
from concourse.bass_utils import run_bass_kernel_spmd
from concourse.dve_spec import AluOp, C0, Spec, Src0, Src1, lower, scan
from concourse.dve_uop import DveOpSpec
from concourse.masks import make_identity

F32 = mybir.dt.float32
F32R = mybir.dt.float32r
N_CORES = 8
E_TOTAL = 32768
E_CORE = E_TOTAL // N_CORES
N_NODES = 16384
RES_DIM = 56
H = 64
W_IN = 80
W_NUMEL = 5120
EPS = 1e-5
CHUNK_PAGES = 16  # h-pages (of 80 values) per PSUM chunk
PRELOAD = 1


def _register_mul_scan():
    name = "MUL_SCAN_ANT"
    for op in dve_ops.OPS:
        if op.name == name:
            return op
    spec = Spec(
        body=scan(AluOp.ADD, Src0 * Src1, init=C0),
        reference=lambda in0, in1, s0, *a: (
            np.asarray(s0, np.float32)
            + np.cumsum(
                (in0.reshape(in0.shape[0], -1)
                 * in1.reshape(in1.shape[0], -1)).astype(np.float32),
                axis=-1)).astype(np.float32),
    )
    opcode = dve_ops._CUSTOM_DVE_ROW_BASE + len(dve_ops.OPS)
    shas = {}
    for ver in ("v3", "v4"):
        shas[ver] = DveOpSpec(name=name, opcode=opcode,
                              uops=lower(spec, ver=ver), rd1_en=True).sha(ver)
    op = dve_ops.DveOp(name, spec, subdim=False, uops_sha=shas)
    dve_ops.OPS.append(op)
    dve_ops.CUSTOM_DVE_SPECS[name] = spec
    dve_ops._SUB_OPCODE_FOR_NAME[name] = opcode
    return op


MUL_SCAN = _register_mul_scan()


def _build_kernel():
    n_tiles = E_CORE // 128
    chunks = []
    h = 0
    while h < H:
        n = min(CHUNK_PAGES, H - h)
        chunks.append((h, n))
        h += n

    nc = bacc.Bacc("TRN2", target_bir_lowering=False, debug=False,
                   enable_asserts=False, num_devices=N_CORES)

    efsi_ap = nc.dram_tensor("efsi", [E_CORE, H + 6], F32,
                             kind="ExternalInput").ap()
    eft_ap = nc.dram_tensor("eft", [H, E_CORE], F32, kind="ExternalInput").ap()
    res_ap = nc.dram_tensor("res", [N_NODES, RES_DIM], F32,
                            kind="ExternalInput").ap()
    w1_ap = nc.dram_tensor("w1", [H, 128], F32, kind="ExternalInput").ap()
    b1_ap = nc.dram_tensor("b1", [128, 1], F32, kind="ExternalInput").ap()
    w2_ap = nc.dram_tensor("w2", [128, W_NUMEL], F32R, kind="ExternalInput").ap()
    b2r_ap = nc.dram_tensor("b2r", [W_IN, H], F32, kind="ExternalInput").ap()
    gb_ap = nc.dram_tensor("gb", [2, H], F32, kind="ExternalInput").ap()
    out_ap = nc.dram_tensor("out", [E_CORE, H], F32, kind="ExternalOutput").ap()

    with tile.TileContext(nc) as tc, ExitStack() as ctx:
        singles = ctx.enter_context(tc.tile_pool(name="singles", bufs=1))
        loads = ctx.enter_context(tc.tile_pool(name="loads", bufs=3))
        mids = ctx.enter_context(tc.tile_pool(name="mids", bufs=3))
        outs = ctx.enter_context(tc.tile_pool(name="outs", bufs=3))
        ps_small = ctx.enter_context(
            tc.tile_pool(name="ps_small", bufs=1, space="PSUM"))
        ps_rh = ctx.enter_context(
            tc.tile_pool(name="ps_rh", bufs=1, space="PSUM"))
        ps_w = ctx.enter_context(tc.tile_pool(name="ps_w", bufs=2, space="PSUM"))

        # --- resident constants ---
        w1sb = singles.tile([H, 128], F32)
        nc.sync.dma_start(w1sb[:], w1_ap[:])
        b1sb = singles.tile([128, 1], F32)
        nc.sync.dma_start(b1sb[:], b1_ap[:])
        b2rsb = singles.tile([W_IN, H], F32)
        nc.sync.dma_start(b2rsb[:], b2r_ap[:])
        gammab = singles.tile([128, H], F32)
        nc.sync.dma_start(gammab[:], bass.AP(
            tensor=gb_ap.tensor, offset=gb_ap.offset,
            ap=[[0, 128]] + gb_ap[0:1, :].ap[1:]))
        betab = singles.tile([128, H], F32)
        nc.sync.dma_start(betab[:], bass.AP(
            tensor=gb_ap.tensor, offset=gb_ap.offset + H,
            ap=[[0, 128]] + gb_ap[1:2, :].ap[1:]))
        # scheduling nudge: one extra sync-queue DMA here shifts the DMA
        # queue interleave so tile-0's critical transfers land ~3us earlier
        scratch = singles.tile([1, W_NUMEL], F32R)
        nc.sync.dma_start(scratch[:], w2_ap[0:1, :])
        epsb = singles.tile([128, 1], F32)
        nc.vector.memset(epsb[:], EPS)
        ident = singles.tile([128, 128], F32)
        make_identity(nc, ident[:])

        w2blocks = []
        nblk = 4
        blkw = W_NUMEL // nblk

        def load_w2(blocks):
            for bi in blocks:
                w2b = singles.tile([128, blkw], F32R, tag=f"w2b{bi}")
                nc.scalar.dma_start(w2b[:], w2_ap[:, bi * blkw:(bi + 1) * blkw])
                w2blocks.append(w2b)

        def w2_slice(c0, c1):
            bi = c0 // blkw
            assert c1 <= (bi + 1) * blkw
            return w2blocks[bi][:, c0 - bi * blkw:c1 - bi * blkw]

        def emit_loads(it, eng=None):
            eng = eng or nc.sync
            rows = slice(it * 128, (it + 1) * 128)
            efsi = loads.tile([128, H + 6], F32, tag="efsi")
            eng.dma_start(efsi[:], efsi_ap[rows, :])
            ef = efsi[:, 0:H]
            sh = efsi[:, H:H + 4]
            idx = efsi[:, H + 4:H + 6].bitcast(mybir.dt.int32)
            src = loads.tile([128, RES_DIM], F32, tag="gath")
            dst = loads.tile([128, RES_DIM], F32, tag="gath")
            nc.gpsimd.indirect_dma_start(
                out=src[:], out_offset=None, in_=res_ap[:],
                in_offset=bass.IndirectOffsetOnAxis(ap=idx[:, 0:1], axis=0))
            nc.gpsimd.indirect_dma_start(
                out=dst[:], out_offset=None, in_=res_ap[:],
                in_offset=bass.IndirectOffsetOnAxis(ap=idx[:, 1:2], axis=0))
            efT = mids.tile([H, 128], F32, tag="efT")
            eng.dma_start(efT[:], eft_ap[:, rows])
            return ef, sh, idx, src, dst, efT

        def emit_tail(rows, S, x_pre):
            x = outs.tile([128, H], F32, tag="x")
            nc.vector.tensor_tensor(out=x[:], in0=x_pre[:], in1=S[:],
                                    op=mybir.AluOpType.add)
            nc.vector.tensor_tensor(out=x[:, 1:], in0=x[:, 1:],
                                    in1=S[:, :H - 1],
                                    op=mybir.AluOpType.subtract)
            stats = outs.tile([128, 6], F32, tag="stats")
            nc.vector.bn_stats(out=stats[:], in_=x[:])
            mv = outs.tile([128, 2], F32, tag="mv")
            nc.vector.bn_aggr(out=mv[:], in_=stats[:])
            std = outs.tile([128, 1], F32, tag="std")
            nc.scalar.activation(std[:], mv[:, 1:2],
                                 mybir.ActivationFunctionType.Sqrt,
                                 bias=epsb[:], scale=1.0)
            rstd = outs.tile([128, 1], F32, tag="rstd")
            nc.vector.reciprocal(rstd[:], std[:])
            y = outs.tile([128, H], F32, tag="y")
            nc.gpsimd.tensor_scalar(out=y[:], in0=x[:], scalar1=mv[:, 0:1],
                                    scalar2=rstd[:],
                                    op0=mybir.AluOpType.subtract,
                                    op1=mybir.AluOpType.mult)
            nc.gpsimd.tensor_tensor(out=y[:], in0=y[:], in1=gammab[:],
                                    op=mybir.AluOpType.mult)
            nc.gpsimd.tensor_tensor(out=y[:], in0=y[:], in1=betab[:],
                                    op=mybir.AluOpType.add)
            nc.sync.dma_start(out_ap[rows, :], y[:])

        load_w2([0])
        # preload the first tile's inputs via the Pool SWDGE path with top
        # priority so they don't queue behind the W2 spray on the HW queues
        with tc.high_priority():
            pre = {it: emit_loads(it, nc.gpsimd)
                   for it in range(min(PRELOAD, n_tiles))}
        load_w2([1, 2, 3])
        pending = None

        for it in range(n_tiles):
            rows = slice(it * 128, (it + 1) * 128)
            ef, sh, idx, src, dst, efT = (
                pre[it] if it in pre else emit_loads(it))

            # --- FFN1, transposed out: rhT = relu(W1.T @ efT + b1) ---
            rhT_ps = ps_rh.tile([128, 128], F32, tag="psr")
            nc.tensor.matmul(rhT_ps[:], lhsT=w1sb[:], rhs=efT[:],
                             start=True, stop=True)
            rhT = mids.tile([128, 128], F32R)
            nc.scalar.activation(rhT[:], rhT_ps[:],
                                 mybir.ActivationFunctionType.Relu,
                                 bias=b1sb[:], scale=1.0)

            # --- feats [128, 80] ---
            feats = mids.tile([128, W_IN], F32)
            nc.scalar.activation(feats[:, 0:32], src[:, 0:32],
                                 mybir.ActivationFunctionType.Copy,
                                 scale=sh[:, 0:1])
            nc.scalar.activation(feats[:, 32:64], dst[:, 0:32],
                                 mybir.ActivationFunctionType.Copy,
                                 scale=sh[:, 0:1])
            vtmp = mids.tile([128, 8], F32)
            for half, g in ((src, slice(64, 72)), (dst, slice(72, 80))):
                vecs = half[:, 32:RES_DIM].rearrange("p (m c) -> p m c", c=3)
                nc.gpsimd.tensor_scalar_mul(feats[:, g], in0=vecs[:, :, 0],
                                            scalar1=sh[:, 1:2])
                for comp in (1, 2):
                    nc.gpsimd.tensor_scalar_mul(
                        vtmp[:], in0=vecs[:, :, comp],
                        scalar1=sh[:, 1 + comp:2 + comp])
                    nc.gpsimd.tensor_tensor(out=feats[:, g], in0=feats[:, g],
                                            in1=vtmp[:],
                                            op=mybir.AluOpType.add)

            # --- b2 seed: feats @ b2r via PE transpose of feats ---
            def emit_seed(feats, ef):
                featsT_ps = ps_small.tile([W_IN, 128], F32, tag="ps")
                nc.tensor.transpose(featsT_ps[:], feats[:], ident[:])
                featsT = mids.tile([W_IN, 128], F32, tag="featsT")
                nc.scalar.copy(featsT[:], featsT_ps[:])
                seed_ps = ps_small.tile([128, H], F32, tag="ps")
                nc.tensor.matmul(seed_ps[:], lhsT=featsT[:], rhs=b2rsb[:],
                                 start=True, stop=True)
                seedc = mids.tile([128, H], F32, tag="seedc")
                nc.scalar.copy(seedc[:], seed_ps[:])
                x_pre = outs.tile([128, H], F32, tag="x_pre")
                nc.gpsimd.tensor_tensor(out=x_pre[:], in0=ef[:], in1=seedc[:],
                                        op=mybir.AluOpType.add)
                return x_pre

            # for tile 0, defer the seed past the scans so the PE FIFO
            # reaches the first FFN2 matmul without feats-chain waits
            if it > 0:
                x_pre = emit_seed(feats, ef)

            # --- FFN2 (f32r) + fused scan contraction into S ---
            S = outs.tile([128, H], F32, tag="S")
            for ci, (h0, npages) in enumerate(chunks):
                width = npages * W_IN
                w_ps = ps_w.tile([128, width], F32, tag="psw")
                col = 0
                while col < width:
                    n = min(512, width - col)
                    nc.tensor.matmul(
                        w_ps[:, col:col + n], lhsT=rhT[:],
                        rhs=w2_slice(h0 * W_IN + col, h0 * W_IN + col + n),
                        start=True, stop=True)
                    col += n
                init = 0.0 if ci == 0 else S[:, h0 - 1:h0]
                feats_b = bass.AP(tensor=feats[:].tensor,
                                  offset=feats[:].offset,
                                  ap=[feats[:].ap[0], [0, npages], [1, W_IN]])
                s_out = bass.AP(tensor=S[:].tensor, offset=S[:].offset + h0,
                                ap=[S[:].ap[0], [1, npages], [0, W_IN]])
                nc.vector._custom_dve(
                    MUL_SCAN, out=s_out,
                    in0=w_ps[:].rearrange("p (s n) -> p s n", n=W_IN),
                    in1=feats_b, s0=init)

            if it == 0:
                x_pre = emit_seed(feats, ef)
            if pending is not None:
                emit_tail(*pending)
            pending = (rows, S, x_pre)

        if pending is not None:
            emit_tail(*pending)

    nc.compile()
    return nc


_NC_CACHE = None


def _get_nc():
    global _NC_CACHE
    if _NC_CACHE is None:
        _NC_CACHE = _build_kernel()
    return _NC_CACHE


def _host_prep(inputs):
    ef = np.asarray(inputs["edge_features"], np.float32)
    sh = np.asarray(inputs["edge_sh"], np.float32).copy()
    sh[:, 1:4] /= np.float32(np.sqrt(3.0))
    idx = np.asarray(inputs["edge_index"])
    res = np.ascontiguousarray(np.asarray(inputs["res_features"], np.float32))
    w1 = np.ascontiguousarray(np.asarray(inputs["W1"], np.float32))
    b1 = np.ascontiguousarray(
        np.asarray(inputs["b1"], np.float32).reshape(128, 1))
    scale = np.float32(1.0 / np.sqrt(80.0))
    w2 = np.asarray(inputs["W2"], np.float32) * scale
    # h-major permutation: col h*80+d = w2[:, d*64+h]
    w2 = np.ascontiguousarray(
        w2.reshape(128, W_IN, H).transpose(0, 2, 1).reshape(128, W_NUMEL))
    b2r = np.ascontiguousarray(
        (np.asarray(inputs["b2"], np.float32) * scale).reshape(W_IN, H))
    gb = np.ascontiguousarray(np.stack([
        np.asarray(inputs["gamma"], np.float32),
        np.asarray(inputs["beta"], np.float32)]))
    # idx cols: 0 = src (edge_index[1]), 1 = dst (edge_index[0]), as int32
    idx2 = np.stack([idx[1], idx[0]], axis=1).astype(np.int32)

    in_maps = []
    for c in range(N_CORES):
        rows = slice(c * E_CORE, (c + 1) * E_CORE)
        efsi = np.concatenate(
            [ef[rows], sh[rows], idx2[rows].view(np.float32)], axis=1)
        in_maps.append(dict(
            efsi=np.ascontiguousarray(efsi),
            eft=np.ascontiguousarray(ef[rows].T),
            res=res, w1=w1, b1=b1, w2=w2, b2r=b2r, gb=gb,
        ))
    return in_maps


def kernel(**inputs) -> np.ndarray:
    assert inputs["edge_features"].shape == (E_TOTAL, H)
    nc = _get_nc()
    in_maps = _host_prep(inputs)
    res = run_bass_kernel_spmd(nc, in_maps, core_ids=list(range(N_CORES)))
    return np.concatenate([r["out"] for r in res.results], axis=0)
